# revision 1
# baseline (speedup 1.0000x reference)
"""CFD-GCN Trainium2 kernel: 6-layer GCN on a batched random mesh graph +
KNN interpolation, distributed over 8 NeuronCores (one sample per core pair).

Self-contained: hardcodes all shapes. kernel(**inputs) -> np.ndarray [80000, 3].
"""
import sys

sys.path.insert(0, "/opt/trn_rl_repo")

import numpy as np
import ml_dtypes

from concourse import bass, bacc
from concourse.bass_utils import run_bass_kernel_spmd
import concourse.mybir as mybir
from contextlib import ExitStack

f32, bf16 = mybir.dt.float32, mybir.dt.bfloat16
i16, u16 = mybir.dt.int16, mybir.dt.uint16
ALU = mybir.AluOpType
ACTF = mybir.ActivationFunctionType
bfnp = ml_dtypes.bfloat16

# ---------------- problem constants ----------------
B, NF, NC, H, D_IN, OUT = 4, 20000, 2000, 512, 5, 3
E_PER = 6 * NF
NT = 158                      # node tiles per sample
NPAD = NT * 128               # 20224
NCPAD = 2048                  # padded coarse count
SLICES = 8                    # 128-slot edge slices per dest tile (uniform)
TSLOTS = SLICES * 128         # 1024 slots per tile
ECAP = NT * TSLOTS            # 161792 edge slots per sample
RANGE_T = 16                  # node tiles per hT transpose-load range
ICH = 8                       # interp gather chunk (tiles)
N_CORES = 8
PHASE = 99                    # debug: truncate program after checkpoint N

LAYERS = [
    dict(kc6=True, fy=False, relu=True, e2=False),   # pre0
    dict(kc6=False, fy=False, relu=True, e2=False),  # pre1
    dict(kc6=False, fy=False, relu=True, e2=False),  # pre2
    dict(kc6=False, fy=True, relu=True, e2=False),   # end0
    dict(kc6=False, fy=False, relu=True, e2=False),  # end1
    dict(kc6=False, fy=False, relu=False, e2=True),  # end2
]


def _ranges():
    r, t0 = [], 0
    while t0 < NT:
        r.append((t0, min(RANGE_T, NT - t0)))
        t0 += RANGE_T
    return r


def build_program():
    nc = bacc.Bacc()

    Din = {}
    def din(name, shape, dt):
        Din[name] = nc.declare_dram_parameter(name, list(shape), dt, isOutput=False)
    def dout(name, shape, dt):
        Din[name] = nc.declare_dram_parameter(name, list(shape), dt, isOutput=True)

    din("xT3", (3, NPAD), f32)
    din("cxT3", (3, NCPAD), f32)
    din("negf2", (128, NT), f32)
    din("h0T", (6, NPAD), bf16)
    din("W0", (6, H), bf16)
    din("W1", (128, 4, H), bf16)      # p-major k-chunked
    din("W2", (128, 4, H), bf16)
    din("W3a", (128, 4, H), bf16)
    din("W3b", (3, H), bf16)
    din("W4", (128, 4, H), bf16)
    din("W5", (128, 4, 128), bf16)
    din("brows", (1, 6, H), bf16)
    din("ones1", (1, 128), bf16)
    din("identb", (128, 128), bf16)
    din("sTw", (128, ECAP // 128, 128), bf16)
    din("groww", (128, ECAP // 16), i16)
    din("ctab", (NCPAD, 128), bf16)

    g_d = nc.dram_tensor("g_d", [NPAD, H], bf16)
    fy_d = nc.dram_tensor("fy_d", [3, NPAD], bf16)
    g2_d = nc.dram_tensor("g2_d", [NPAD, 128], bf16)
    h_d = nc.dram_tensor("h_d", [NPAD, H], bf16)
    dout("out_nm", (NPAD, 128), f32)

    es = ExitStack()
    def sb(name, shape, dt):
        return es.enter_context(nc.sbuf_tensor(name, list(shape), dt))
    def psum(name, shape, dt):
        return es.enter_context(nc.psum_tensor(name, list(shape), dt))

    xt_s = [sb(f"xt_s{i}", (3, 128), f32) for i in range(2)]
    cxT3_s = sb("cxT3_s", (3, NCPAD), f32)
    negf2_s = sb("negf2_s", (128, NT), f32)
    h0_s = [sb(f"h0_s{i}", (6, 128), bf16) for i in range(2)]
    W0_s = sb("W0_s", (6, H), bf16)
    W1_s = sb("W1_s", (128, 4, H), bf16)
    W2_s = sb("W2_s", (128, 4, H), bf16)
    W3a_s = sb("W3a_s", (128, 4, H), bf16)
    W3b_s = sb("W3b_s", (3, H), bf16)
    W4_s = sb("W4_s", (128, 4, H), bf16)
    W5_s = sb("W5_s", (128, 4, 128), bf16)
    brows_s = sb("brows_s", (1, 6, H), bf16)
    ones1_s = sb("ones1_s", (1, 128), bf16)
    identb_s = sb("identb_s", (128, 128), bf16)
    gro_s = [sb(f"gro_s{i}", (128, 128), i16) for i in range(2)]

    hT_s = [sb(f"hT_s{i}", (128, 4, RANGE_T * 128), bf16) for i in range(2)]
    gsb_s = [sb(f"gsb_s{i}", (128, H), bf16) for i in range(4)]
    hsb_s = [sb(f"hsb_s{i}", (128, H), bf16) for i in range(4)]
    osb_s = [sb(f"osb_s{i}", (128, 128), f32) for i in range(2)]
    gath_s = [sb(f"gath_s{i}", (128, 16, H), bf16) for i in range(2)]
    gath2_s = [sb(f"gath2_s{i}", (128, 16, 128), bf16) for i in range(2)]
    sT_s = [sb(f"sT_s{i}", (128, 16, 128), bf16) for i in range(2)]

    nd2_s = [sb(f"nd2_s{i}", (128, NCPAD), f32) for i in range(2)]
    bm_s = sb("bm_s", (128, 8, NT), f32)
    bi_s = sb("bi_s", (128, 8, NT), u16)
    d2c_s = sb("d2c_s", (128, 3, NT), f32)
    w_s = sb("w_s", (128, 3, NT), f32)
    wsum_s = sb("wsum_s", (128, NT), f32)
    rs_s = sb("rs_s", (128, NT), f32)
    wnb_s = sb("wnb_s", (128, 3, NT), f32)
    wrap_s = sb("wrap_s", (128, 3, NT, 8), u16)
    gk_s = [[sb(f"gk_s{k}_{i}", (128, ICH, 128), bf16) for i in range(2)]
            for k in range(3)]
    diag_s = [sb(f"diag_s{i}", (128, 128), bf16) for i in range(6)]
    fyw_s = [sb(f"fyw_s{i}", (3, 128), bf16) for i in range(2)]
    fyr_s = [sb(f"fyr_s{i}", (3, 128), bf16) for i in range(2)]

    pz = [psum(f"pz{i}", (128, H), f32) for i in range(2)]
    pa = [psum(f"pa{i}", (128, H), f32) for i in range(2)]

    class Sem:
        def __init__(self, name):
            self.h = es.enter_context(nc.semaphore(name))
            self.n = 0
        def inc(self, k):
            self.n += k
            return (self.h, self.n)
        def now(self):
            return (self.h, self.n)

    class Ring:
        def __init__(self, name, n):
            self.sems = [Sem(f"{name}{i}") for i in range(n)]
            self.nslots = n
        def write(self, slot, k=16):
            s = self.sems[slot % self.nslots]
            return s.inc(k)
        def last(self, slot):
            s = self.sems[slot % self.nslots]
            return (s.h, s.n)
        def all(self):
            return [(s.h, s.n) for s in self.sems]

    def wait_all(engine, ring):
        for sv in ring.all():
            wait(engine, sv)

    s_in = Sem("s_in")
    s_kpe = Sem("s_kpe"); s_kact = Sem("s_kact"); s_kmax = Sem("s_kmax")
    s_wn = Sem("s_wn"); s_wrap = Sem("s_wrap")
    s_dg = Sem("s_dg"); s_ipe = Sem("s_ipe"); s_fy = Sem("s_fy")
    s_dpe = Sem("s_dpe"); s_zpe = Sem("s_zpe")
    s_zact = Sem("s_zact"); s_ape = Sem("s_ape"); s_aact = Sem("s_aact")
    r_gk = None  # created below


    Q = {e: [] for e in ("sync", "tensor", "vector", "scalar", "gpsimd")}
    checkpoints = []
    def checkpoint():
        checkpoints.append({e: len(Q[e]) for e in Q})
    def emit(engine, fn):
        Q[engine].append(fn)
    def wait(engine, semv):
        s, v = semv
        if v > 0:
            emit(engine, lambda e, s=s, v=v: e.wait_ge(s, v))

    r_gk = Ring("r_gk", 2)     # interp table gathers (per gk buf)
    r_xt = Ring("r_xt", 2)     # xT3 tile loads
    r_h0 = Ring("r_h0", 2)     # h0T tile loads
    r_gro = Ring("r_gro", 2)   # gather idx chunk loads
    r_fyw = Ring("r_fyw", 2)   # finey dram writes
    r_fyr = Ring("r_fyr", 2)   # finey tile loads
    r_hT = Ring("r_hT", 2)     # transpose loads (per hT buf)
    r_g = Ring("r_g", 2)       # agg gathers (per gath buf)
    r_s = Ring("r_s", 2)       # S loads (per sT buf)
    r_gw = Ring("r_gw", 4)     # g dram writes (per gsb buf)
    r_hw = Ring("r_hw", 4)     # h dram writes (per hsb buf)
    r_ow = Ring("r_ow", 2)     # out writes (per osb buf)

    # ============ input loads ============
    loads = [
        (cxT3_s[:], "cxT3"), (negf2_s[:], "negf2"),
        (W0_s[:], "W0"), (W1_s[:], "W1"), (W2_s[:], "W2"),
        (W3a_s[:], "W3a"), (W3b_s[:], "W3b"), (W4_s[:], "W4"), (W5_s[:], "W5"),
        (brows_s[:], "brows"), (ones1_s[:], "ones1"), (identb_s[:], "identb"),
    ]
    for dst, srcn in loads:
        sm = s_in.inc(16)
        emit("sync", lambda e, d=dst, s=srcn, sm=sm: e.dma_start(
            out=d, in_=Din[s][:]).then_inc(sm[0], 16))
    IN_ALL = s_in.now()
    checkpoint()   # 0: loads

    # ============ KNN selection ============
    wait("tensor", IN_ALL)
    wait("scalar", IN_ALL)
    wait("vector", IN_ALL)
    NQ = NCPAD // 512
    for t in range(NT):
        if t >= 2:
            wait("sync", (s_kpe.h, NQ * (t - 1)))
        sm = r_xt.write(t)
        emit("sync", lambda e, t=t, sm=sm: e.dma_start(
            out=xt_s[t % 2][:], in_=Din["xT3"][:, t * 128:(t + 1) * 128]
        ).then_inc(sm[0], 16))
        wait("tensor", r_xt.last(t))
        for q in range(NQ):
            gq = NQ * t + q
            if gq >= 2:
                wait("tensor", (s_kact.h, gq - 1))
            sm = s_kpe.inc(1)
            emit("tensor", lambda e, t=t, q=q, gq=gq, sm=sm: e.matmul(
                pz[gq % 2][:, 0:512], xt_s[t % 2][:],
                cxT3_s[:, q * 512:(q + 1) * 512],
                start=True, stop=True).then_inc(sm[0], 1))
        for q in range(NQ):
            gq = NQ * t + q
            wait("scalar", (s_kpe.h, gq + 1))
            if t >= 2 and q == 0:
                wait("scalar", (s_kmax.h, t - 1))
            sm = s_kact.inc(1)
            emit("scalar", lambda e, t=t, q=q, gq=gq, sm=sm: e.activation(
                nd2_s[t % 2][:, q * 512:(q + 1) * 512], pz[gq % 2][:, 0:512],
                ACTF.Identity, bias=negf2_s[:, t:t + 1], scale=1.0
            ).then_inc(sm[0], 1))
        wait("vector", (s_kact.h, NQ * (t + 1)))
        emit("vector", lambda e, t=t: e.max(bm_s[:, :, t], nd2_s[t % 2][:]))
        emit("vector", lambda e: e.drain())
        emit("vector", lambda e, t=t: e.max_index(
            bi_s[:, :, t], bm_s[:, :, t], nd2_s[t % 2][:]))
        sm = s_kmax.inc(1)
        emit("vector", lambda e, sm=sm: e.drain().then_inc(sm[0], 1))

    checkpoint()   # 1: knn select
    # weights on DVE
    emit("vector", lambda e: e.tensor_scalar(
        out=d2c_s[:], in0=bm_s[:, 0:3, :], scalar1=-1.0, scalar2=1e-16,
        op0=ALU.mult, op1=ALU.max))
    emit("vector", lambda e: e.drain())
    emit("vector", lambda e: e.reciprocal(w_s[:], d2c_s[:]))
    emit("vector", lambda e: e.drain())
    emit("vector", lambda e: e.tensor_reduce(
        out=wsum_s[:], in_=bass.AP(w_s, 0, [[3 * NT, 128], [1, NT], [NT, 3]]),
        axis=mybir.AxisListType.X, op=ALU.add))
    emit("vector", lambda e: e.drain())
    emit("vector", lambda e: e.reciprocal(rs_s[:], wsum_s[:]))
    emit("vector", lambda e: e.drain())
    emit("vector", lambda e: e.tensor_tensor(
        out=wnb_s[:], in0=w_s[:],
        in1=bass.AP(rs_s, 0, [[NT, 128], [0, 3], [1, NT]]),
        op=ALU.mult))
    sm = s_wn.inc(1)
    emit("vector", lambda e, sm=sm: e.drain().then_inc(sm[0], 1))

    # wrapped idx build (gpsimd)
    wait("gpsimd", (s_kmax.h, NT))
    for k in range(3):
        for g in range(8):
            sm = s_wrap.inc(16)
            emit("gpsimd", lambda e, k=k, g=g, sm=sm: e.dma_start(
                out=wrap_s[0:16, k, :, g],
                in_=bi_s[16 * g:16 * (g + 1), k, :],
            ).then_inc(sm[0], 16))
    wait("gpsimd", s_wrap.now())
    for rep in range(1, 8):
        sm = s_wrap.inc(16)
        emit("gpsimd", lambda e, rep=rep, sm=sm: e.dma_start(
            out=wrap_s[16 * rep:16 * (rep + 1)],
            in_=wrap_s[0:16],
        ).then_inc(sm[0], 16))
    WRAP_ALL = s_wrap.now()

    # interp
    wait("gpsimd", WRAP_ALL)
    wait("vector", s_wn.now())
    n_ich = (NT + ICH - 1) // ICH
    for c in range(n_ich):
        t0 = c * ICH
        ntile = min(ICH, NT - t0)
        if c >= 2:
            wait("gpsimd", (s_ipe.h, (c - 1) * ICH))
        for k in range(3):
            sm = r_gk.write(c)
            emit("gpsimd", lambda e, k=k, c=c, t0=t0, nt=ntile, sm=sm:
                 e.dma_gather(
                     out_ap=gk_s[k][c % 2][:, 0:nt, :],
                     in_ap=Din["ctab"][:],
                     idxs_ap=wrap_s[:, k, t0:t0 + nt, :].bitcast(i16),
                     num_idxs=nt * 128, num_idxs_reg=nt * 128,
                     elem_size=128,
                 ).then_inc(sm[0], 16))
        GK_NOW = r_gk.last(c)
        for tt in range(ntile):
            t = t0 + tt
            if t >= 2:
                wait("vector", (s_ipe.h, t - 1))
            for k in range(3):
                emit("vector", lambda e, t=t, k=k: e.tensor_scalar(
                    out=diag_s[(3 * t + k) % 6][:], in0=identb_s[:],
                    scalar1=wnb_s[:, k, t:t + 1], scalar2=None, op0=ALU.mult))
            sm = s_dg.inc(3)
            emit("vector", lambda e, sm=sm: e.drain().then_inc(sm[0], 3))
            wait("tensor", GK_NOW)
            wait("tensor", (s_dg.h, s_dg.n))
            wait("tensor", (s_fy.h, t))      # psum WAR (skipped when 0)
            for k in range(3):
                sm = s_ipe.inc(1) if k == 2 else None
                def mk_interp(t=t, tt=tt, k=k, c=c, sm=sm):
                    def f(e):
                        ins = e.matmul(
                            pa[0][:, 0:128], gk_s[k][c % 2][:, tt, :],
                            diag_s[(3 * t + k) % 6][:],
                            start=(k == 0), stop=(k == 2))
                        if sm:
                            ins.then_inc(sm[0], 1)
                    return f
                emit("tensor", mk_interp())
            wait("scalar", (s_ipe.h, s_ipe.n))
            wait("scalar", r_fyw.last(t))
            sm = s_fy.inc(1)
            emit("scalar", lambda e, t=t, sm=sm: e.activation(
                fyw_s[t % 2][:], pa[0][0:3, 0:128],
                ACTF.Copy, bias=0.0, scale=1.0).then_inc(sm[0], 1))
            wait("sync", (s_fy.h, s_fy.n))
            sm = r_fyw.write(t)
            emit("sync", lambda e, t=t, sm=sm: e.dma_start(
                out=fy_d[:, t * 128:(t + 1) * 128],
                in_=fyw_s[t % 2][:]).then_inc(sm[0], 16))
    FY_ALL = s_fy.now()
    KACT_ALL = s_kact.now()
    checkpoint()   # 2: interp

    # ============ GCN layers ============
    WCH = {1: W1_s, 2: W2_s, 3: W3a_s, 4: W4_s, 5: W5_s}

    for li, L in enumerate(LAYERS):
        width = 128 if L["e2"] else H
        gdst = g2_d if L["e2"] else g_d

        # ---------- dense ----------
        zpe_base = s_zpe.n
        zact_base = s_zact.n

        def dense_epilogue(t):
            wait("scalar", (s_zpe.h, zpe_base + t + 1))
            wait("scalar", r_gw.last(t))
            sm = s_zact.inc(1)
            emit("scalar", lambda e, t=t, w=width, sm=sm: e.activation(
                gsb_s[t % 4][:, 0:w], pz[t % 2][:, 0:w], ACTF.Copy,
                bias=0.0, scale=1.0).then_inc(sm[0], 1))
            wait("sync", (s_zact.h, s_zact.n))
            sm = r_gw.write(t)
            emit("sync", lambda e, t=t, gd=gdst, w=width, sm=sm: e.dma_start(
                out=gd[t * 128:(t + 1) * 128, :],
                in_=gsb_s[t % 4][:, 0:w]).then_inc(sm[0], 16))

        if li == 0:
            wait("tensor", KACT_ALL)      # pz WAR vs KNN ACT
            for t in range(NT):
                if t >= 2:
                    wait("sync", (s_zpe.h, zpe_base + t - 1))
                sm = r_h0.write(t)
                emit("sync", lambda e, t=t, sm=sm: e.dma_start(
                    out=h0_s[t % 2][:], in_=Din["h0T"][:, t * 128:(t + 1) * 128]
                ).then_inc(sm[0], 16))
                wait("tensor", r_h0.last(t))
                wait("tensor", (s_zact.h, zact_base if t < 2 else zact_base + t - 1))
                sm = s_zpe.inc(1)
                emit("tensor", lambda e, t=t, sm=sm: e.matmul(
                    pz[t % 2][:, 0:H], h0_s[t % 2][:],
                    W0_s[:], start=True, stop=True).then_inc(sm[0], 1))
                dense_epilogue(t)
        else:
            Wl = WCH[li]
            range_zpe = []
            for ri, (rt0, rnt) in enumerate(_ranges()):
                wait("sync", (s_zpe.h,
                              zpe_base if ri < 2 else range_zpe[ri - 2]))
                for cch in range(4):
                    sm = r_hT.write(ri)
                    emit("sync", lambda e, ri=ri, rt0=rt0, rnt=rnt, c=cch, sm=sm:
                         e.dma_start_transpose(
                             hT_s[ri % 2][:, c, 0:rnt * 128],
                             h_d[rt0 * 128:(rt0 + rnt) * 128,
                                 c * 128:(c + 1) * 128],
                         ).then_inc(sm[0], 16))
                wait("tensor", r_hT.last(ri))
                if li == 3 and ri == 0:
                    wait_all("sync", r_fyw)
                for tt in range(rnt):
                    t = rt0 + tt
                    range_last = (tt == rnt - 1)
                    if L["fy"]:
                        if t >= 2:
                            wait("sync", (s_zpe.h, zpe_base + t - 1))
                        sm = r_fyr.write(t)
                        emit("sync", lambda e, t=t, sm=sm: e.dma_start(
                            out=fyr_s[t % 2][:],
                            in_=fy_d[:, t * 128:(t + 1) * 128]
                        ).then_inc(sm[0], 16))
                    wait("tensor", (s_zact.h,
                                    zact_base if t < 2 else zact_base + t - 1))
                    for cch in range(4):
                        last = (cch == 3) and not L["fy"]
                        sm = s_zpe.inc(1) if last else None
                        def mk_dense(t=t, tt=tt, ri=ri, cch=cch, Wl=Wl,
                                     w=width, last=last, sm=sm):
                            def f(e):
                                ins = e.matmul(
                                    pz[t % 2][:, 0:w],
                                    hT_s[ri % 2][:, cch,
                                                 tt * 128:(tt + 1) * 128],
                                    Wl[:, cch, 0:w],
                                    start=(cch == 0), stop=last)
                                if sm:
                                    ins.then_inc(sm[0], 1)
                            return f
                        emit("tensor", mk_dense())
                    if L["fy"]:
                        wait("tensor", r_fyr.last(t))
                        sm = s_zpe.inc(1)
                        emit("tensor", lambda e, t=t, sm=sm: e.matmul(
                            pz[t % 2][:, 0:H],
                            fyr_s[t % 2][:],
                            W3b_s[:], start=False, stop=True).then_inc(sm[0], 1))
                    if range_last:
                        range_zpe.append(s_zpe.n)
                    dense_epilogue(t)
        checkpoint()   # dense of this layer done
        # ---------- agg ----------
        gbufs = gath2_s if L["e2"] else gath_s
        ape_base = s_ape.n
        aact_base = s_aact.n
        wait_all("gpsimd", r_gw)
        if li == 0:
            wait("tensor", (s_fy.h, NT))   # pa WAR vs interp
        for c in range(NT // 2):
            wait("gpsimd", (s_ape.h,
                            ape_base if c < 2 else ape_base + 2 * (c - 1)))
            wait("sync", r_g.last(c))
            sm = r_gro.write(c)
            emit("sync", lambda e, c=c, sm=sm: e.dma_start(
                out=gro_s[c % 2][:],
                in_=Din["groww"][:, c * 128:(c + 1) * 128]).then_inc(sm[0], 16))
            wait("gpsimd", r_gro.last(c))
            sm = r_g.write(c)
            emit("gpsimd", lambda e, c=c, gd=gdst, gb=gbufs, w=width, sm=sm:
                 e.dma_gather(
                     out_ap=gb[c % 2][:, :, 0:w],
                     in_ap=gd[:],
                     idxs_ap=gro_s[c % 2][:],
                     num_idxs=2048, num_idxs_reg=2048, elem_size=w,
                     single_packet=False,
                 ).then_inc(sm[0], 16))
            wait("sync", (s_ape.h,
                          ape_base if c < 2 else ape_base + 2 * (c - 1)))
            sm = r_s.write(c)
            emit("sync", lambda e, c=c, sm=sm: e.dma_start(
                out=sT_s[c % 2][:],
                in_=Din["sTw"][:, c * 16:(c + 1) * 16, :]).then_inc(sm[0], 16))
            wait("tensor", r_g.last(c))
            wait("tensor", r_s.last(c))
            for tt in range(2):
                t = 2 * c + tt
                wait("tensor", (s_aact.h,
                                aact_base if t < 2 else aact_base + t - 1))
                for sl in range(SLICES):
                    emit("tensor", lambda e, c=c, tt=tt, t=t, sl=sl, gb=gbufs,
                         w=width: e.matmul(
                        pa[t % 2][:, 0:w],
                        sT_s[c % 2][:, tt * 8 + sl, :],
                        gb[c % 2][:, tt * 8 + sl, 0:w],
                        start=(sl == 0), stop=False))
                sm = s_ape.inc(1)
                emit("tensor", lambda e, t=t, li=li, w=width, sm=sm: e.matmul(
                    pa[t % 2][:, 0:w], ones1_s[:],
                    brows_s[:, li, 0:w], start=False, stop=True
                ).then_inc(sm[0], 1))
                wait("scalar", (s_ape.h, s_ape.n))
                if L["e2"]:
                    wait("scalar", r_ow.last(t))
                else:
                    wait("scalar", r_hw.last(t))
                sm = s_aact.inc(1)
                if L["e2"]:
                    emit("scalar", lambda e, t=t, sm=sm: e.activation(
                        osb_s[t % 2][:], pa[t % 2][:, 0:128], ACTF.Copy,
                        bias=0.0, scale=1.0).then_inc(sm[0], 1))
                else:
                    emit("scalar", lambda e, t=t, sm=sm: e.activation(
                        hsb_s[t % 4][:], pa[t % 2][:, 0:H], ACTF.Relu,
                        bias=0.0, scale=1.0).then_inc(sm[0], 1))
                wait("sync", (s_aact.h, s_aact.n))
                if L["e2"]:
                    sm = r_ow.write(t)
                    emit("sync", lambda e, t=t, sm=sm: e.dma_start(
                        out=Din["out_nm"][t * 128:(t + 1) * 128, :],
                        in_=osb_s[t % 2][:]).then_inc(sm[0], 16))
                else:
                    sm = r_hw.write(t)
                    emit("sync", lambda e, t=t, sm=sm: e.dma_start(
                        out=h_d[t * 128:(t + 1) * 128, :],
                        in_=hsb_s[t % 4][:]).then_inc(sm[0], 16))
        if not L["e2"]:
            wait_all("sync", r_hw)   # barrier before next layer's hT loads
        checkpoint()   # 3+li

    wait_all("sync", r_ow)
    wait_all("sync", r_hw)
    checkpoint()
    if PHASE < len(checkpoints):
        cut = checkpoints[PHASE]
        for e in Q:
            Q[e] = Q[e][:cut[e]]

    with nc.allow_non_contiguous_dma(reason="wrapped idx build"), \
            nc.Block() as block:
        @block.sync
        def _(e):
            for fn in Q["sync"]:
                fn(e)

        @block.tensor
        def _(e):
            for fn in Q["tensor"]:
                fn(e)

        @block.vector
        def _(e):
            for fn in Q["vector"]:
                fn(e)

        @block.scalar
        def _(e):
            for fn in Q["scalar"]:
                fn(e)

        @block.gpsimd
        def _(e):
            for fn in Q["gpsimd"]:
                fn(e)

    nc.finalize()
    return nc


# ================= host side =================

def host_prep(inputs):
    x = np.asarray(inputs["x"], np.float32)
    sdf = np.asarray(inputs["sdf"], np.float32)
    edge_index = np.asarray(inputs["edge_index"], np.int64)
    coarse_x = np.asarray(inputs["coarse_x"], np.float32)
    coarse_y = np.asarray(inputs["coarse_y"], np.float32)
    Ws = {k: np.asarray(inputs[k], np.float32) for k in (
        "pre_W0", "pre_W1", "pre_W2", "end_W0", "end_W1", "end_W2")}
    bs = {k: np.asarray(inputs[k], np.float32) for k in (
        "pre_b0", "pre_b1", "pre_b2", "end_b0", "end_b1", "end_b2")}

    cxT3 = np.zeros((3, NCPAD), np.float32)
    cxT3[0, :NC] = 2 * coarse_x[:, 0]
    cxT3[1, :NC] = 2 * coarse_x[:, 1]
    cxT3[2, :NC] = -(coarse_x[:, 0] ** 2 + coarse_x[:, 1] ** 2)
    cxT3[0, NC:] = 2e4; cxT3[1, NC:] = 2e4; cxT3[2, NC:] = -2e8

    brows = np.zeros((6, H), np.float32)
    for i, k in enumerate(("pre_b0", "pre_b1", "pre_b2", "end_b0", "end_b1")):
        brows[i] = bs[k]
    brows[5, :OUT] = bs["end_b2"]

    W5 = np.zeros((H, 128), np.float32)
    W5[:, :OUT] = Ws["end_W2"]

    def pmaj(w):   # [512, X] -> [128, 4, X]
        return np.ascontiguousarray(
            w.reshape(4, 128, w.shape[1]).transpose(1, 0, 2))

    common = dict(
        cxT3=cxT3,
        W0=Ws["pre_W0"].astype(bfnp),
        W1=pmaj(Ws["pre_W1"]).astype(bfnp),
        W2=pmaj(Ws["pre_W2"]).astype(bfnp),
        W3a=pmaj(Ws["end_W0"][OUT:]).astype(bfnp),
        W3b=Ws["end_W0"][:OUT].astype(bfnp),
        W4=pmaj(Ws["end_W1"]).astype(bfnp),
        W5=pmaj(W5).astype(bfnp),
        brows=brows.astype(bfnp)[None],
        ones1=np.ones((1, 128), bfnp),
        identb=np.eye(128, dtype=np.float32).astype(bfnp),
    )

    in_maps, metas = [], []
    for s in range(B):
        xs = x[s * NF:(s + 1) * NF]
        e = edge_index[:, s * E_PER:(s + 1) * E_PER] - s * NF
        cy = coarse_y[s * NC:(s + 1) * NC]

        deg = np.bincount(e[1], minlength=NF).astype(np.float32) + 1.0
        dinv = (1.0 / np.sqrt(deg)).astype(np.float32)

        # balanced tile assignment (snake over degree-sorted nodes)
        order = np.argsort(-deg, kind="stable")
        tile_seq = np.arange(NT)
        snake = np.concatenate([tile_seq, tile_seq[::-1]])
        bins = np.resize(snake, NF)
        nid = np.empty(NF, np.int64)
        for t in range(NT):
            sel = np.where(bins == t)[0]
            nid[order[sel]] = t * 128 + np.arange(len(sel))

        dinv_new = np.ones(NPAD, np.float32)
        dinv_new[nid] = dinv

        allrow = np.concatenate([nid[e[0]], np.arange(NPAD)])
        allcol = np.concatenate([nid[e[1]], np.arange(NPAD)])
        wts = dinv_new[allrow] * dinv_new[allcol]

        o = np.argsort(allcol, kind="stable")
        allrow, allcol, wts = allrow[o], allcol[o], wts[o]
        tile_of = allcol // 128
        tstart = np.searchsorted(tile_of, np.arange(NT))
        cnts = np.searchsorted(tile_of, np.arange(NT), side="right") - tstart
        assert cnts.max() <= TSLOTS, f"tile overflow {cnts.max()}"

        rank = np.arange(len(allcol)) - np.repeat(tstart, cnts)
        srow = np.zeros((NT, TSLOTS), np.int16)
        srow[tile_of, rank] = allrow.astype(np.int16)
        sT = np.zeros((NT, TSLOTS, 128), np.float32)
        sT[tile_of, rank, allcol % 128] = wts
        sT = sT.reshape(ECAP, 128)
        sTw = np.ascontiguousarray(
            sT.reshape(ECAP // 128, 128, 128).transpose(1, 0, 2)).astype(bfnp)

        grow = srow.reshape(ECAP)
        tmp = np.ascontiguousarray(grow.reshape(ECAP // 16, 16).T)
        groww = np.ascontiguousarray(np.tile(tmp, (8, 1)).astype(np.int16))

        f01 = np.full((NPAD, 2), 1e3, np.float32)
        f01[nid] = xs[:, 0:2]
        xT3 = np.ones((3, NPAD), np.float32)
        xT3[0] = f01[:, 0]; xT3[1] = f01[:, 1]
        negf2 = np.ascontiguousarray(
            (-(f01[:, 0] ** 2 + f01[:, 1] ** 2)).reshape(NT, 128).T)

        h0 = np.zeros((NPAD, 6), np.float32)
        h0[nid, 0:D_IN] = xs
        h0[nid, D_IN] = sdf[:, 0]
        h0T = np.ascontiguousarray(h0.T).astype(bfnp)

        ctab = np.zeros((NCPAD, 128), np.float32)
        ctab[:NC, 0:OUT] = cy
        ctab = ctab.astype(bfnp)

        m = dict(common)
        m.update(xT3=xT3, negf2=negf2, h0T=h0T, sTw=sTw, groww=groww, ctab=ctab)
        in_maps.append(m)
        metas.append(nid)

    full_maps = [in_maps[c // 2] for c in range(N_CORES)]
    return full_maps, metas


_prog_cache = {}
_last_exec_ns = None


def kernel(**inputs):
    global _last_exec_ns
    if "nc" not in _prog_cache:
        _prog_cache["nc"] = build_program()
    nc = _prog_cache["nc"]

    in_maps, metas = host_prep(inputs)
    import os
    trace = bool(os.environ.get("KTRACE"))
    res = run_bass_kernel_spmd(nc, in_maps, list(range(N_CORES)), trace=trace)
    if trace:
        _last_exec_ns = res.exec_time_ns
        _prog_cache["last_res"] = res

    out = np.empty((B * NF, OUT), np.float32)
    for s in range(B):
        o = np.asarray(res.results[2 * s]["out_nm"])
        out[s * NF:(s + 1) * NF] = o[metas[s], 0:OUT]
    return out

